# revision 81
# baseline (speedup 1.0000x reference)
"""Trainium2 Bass kernel for an 8-head MultiHeadAttention (b=8, s=1024, d=512).

Sharding: pure data-parallel over batch -- each of the 8 NeuronCores runs the
full attention for one batch element. No collectives.

Per-core algorithm (matmul operands bf16, accumulate fp32):
  x^T, w^T built via PE transposes.
  Q^T[hd,s] = wq^T.T @ x^T   (scale 1/8 + bias folded into the PSUM drain)
  K^T[hd,s] = wk^T.T @ x^T
  V[s,hd]   = x^T.T @ wv^T   (head-interleaved, ones column per head)
  S^T[k,q]  = K_h^T.T @ Q_h^T  -- head-pair concurrent via PE 64-row tiling:
              even head on partitions 0:64 (tile 0,0), odd head on 64:128
              (tile 64,0); adjacent emission runs the pair concurrently.
  P^T       = exp(S^T) * (1-mask)^T  (exp on ACT; mask-mul split DVE/GPSIMD)
  O^T_h[65,q] = V_aug.T @ P^T  (row 64 = softmax denominator via ones col)
  normalize: drain O^T unnormalized + denominator rows (parked on partitions
              0/32/64/96), one batched DVE reciprocal per head pair (its cost
              is free-dim-bound), indicator-matmul broadcast, in-place mul.
  out[q,d]  = O^T.T @ wo^T + bo

Schedule: software-pipelined at kc granularity. Scores/exp/mask of pair p
interleave with PV matmuls of pairs p-1/p (lagged) plus background work
(V projection, later QK chunks, wo^T build), keeping the ACT-engine exp
chain (the throughput bound) saturated and the tail short. Mask arrives as
8 column strips via SWDGE; warm-up matmuls hold the PE's HAM clock at full
rate through the transpose-heavy ramp.

PSUM banks: psc [128,1024]x3 (ramp, scores, (1-mask)^T, QK chunks, recip
broadcast, final projection) + ppv [128,512]x2 (w^T chunks, V projection,
PV accumulators) = 8.
"""

import numpy as np

P = 128
S = 1024  # sequence length
D = 512  # d_model
H = 8  # heads
DK = 64  # head dim
CH = D // P  # 4 hd/dmodel chunks
ST = S // P  # 8 seq tiles
NCORES = 8

# mask-mul strips handled by gpsimd (per head, by kc index)
GP_MUL_KC = (6, 7)

_CACHE = {}


def _build():
    import concourse.bacc as bacc
    import concourse.mybir as mybir
    import concourse.tile as tile
    from concourse.masks import make_identity

    f32 = mybir.dt.float32
    mmdt = mybir.dt.bfloat16
    AF = mybir.ActivationFunctionType
    OP = mybir.AluOpType

    nc = bacc.Bacc(None, target_bir_lowering=False, debug=False)

    x_t = nc.dram_tensor("x", [S, D], f32, kind="ExternalInput")
    mask_t = nc.dram_tensor("mask", [S, S], f32, kind="ExternalInput")
    wq_t = nc.dram_tensor("wq", [D, D], f32, kind="ExternalInput")
    wk_t = nc.dram_tensor("wk", [D, D], f32, kind="ExternalInput")
    wv_t = nc.dram_tensor("wv", [D, D], f32, kind="ExternalInput")
    wo_t = nc.dram_tensor("wo", [D, D], f32, kind="ExternalInput")
    bq_t = nc.dram_tensor("bq", [D], f32, kind="ExternalInput")
    bk_t = nc.dram_tensor("bk", [D], f32, kind="ExternalInput")
    bv_t = nc.dram_tensor("bv", [D], f32, kind="ExternalInput")
    bo_t = nc.dram_tensor("bo", [D], f32, kind="ExternalInput")
    out_t = nc.dram_tensor("out", [S, D], f32, kind="ExternalOutput")

    with tile.TileContext(nc) as tc:
        with (
            tc.tile_pool(name="persist", bufs=1) as pp,
            tc.tile_pool(name="stage", bufs=1) as stage,
            tc.tile_pool(name="ptp", bufs=4) as ptp,
            tc.tile_pool(name="nrm", bufs=2) as nrm,
            tc.tile_pool(name="fin", bufs=3) as fpool,
            tc.tile_pool(name="psc", bufs=3, space="PSUM") as psc,
            tc.tile_pool(name="ppv", bufs=2, space="PSUM") as ppv,
        ):
            # ---- constants ----
            ident = pp.tile([P, P], f32, name="id", tag="id")
            make_identity(nc, ident[:])
            ones_f32 = pp.tile([P, P], f32, name="ones_f32", tag="ones_f32")
            nc.vector.memset(ones_f32[:], 1.0)
            ones_sb = pp.tile([1, P], mmdt, name="ones", tag="ones")
            nc.vector.tensor_copy(ones_sb[:], ones_f32[0:1, :])
            # indicator for the recip broadcast: denominator slot i lives on
            # partition 32*i; for j-slice, out rows 0:64 take slot 2j and
            # rows 64:128 take slot 2j+1
            e4 = pp.tile([P, 2 * P], mmdt, name="e4", tag="e4")
            nc.vector.memset(e4[:], 0.0)
            for j in range(2):
                nc.vector.memset(
                    e4[32 * 2 * j : 32 * 2 * j + 1, j * P : j * P + 64], 1.0
                )
                nc.vector.memset(
                    e4[32 * (2 * j + 1) : 32 * (2 * j + 1) + 1,
                       j * P + 64 : (j + 1) * P], 1.0
                )

            bq_sb = pp.tile([P, CH], f32, name="bq", tag="bq")
            bk_sb = pp.tile([P, CH], f32, name="bk", tag="bk")
            qbias_sb = pp.tile([P, CH], f32, name="qbias", tag="qbias")

            bv_bc = pp.tile([P, D], f32, name="bvbc", tag="bvbc")
            bo_bc = pp.tile([P, D], f32, name="bobc", tag="bobc")

            # ---- input DMAs: x + weights on the SP HWDGE queue ----
            xc = []
            for c in range(CH):
                t = stage.tile([P, ST, P], f32, name="xc", tag=f"xc{c}")
                nc.sync.dma_start(
                    out=t[:],
                    in_=x_t[:, c * P : (c + 1) * P].rearrange("(i p) d -> p i d", p=P),
                )
                xc.append(t)
            wc = {}

            def dma_w(name, t, eng):
                wc[name] = []
                for c in range(CH):
                    w = stage.tile([P, CH, P], f32, name="wc", tag="wc", bufs=6)
                    eng.dma_start(
                        out=w[:],
                        in_=t[:, c * P : (c + 1) * P].rearrange(
                            "(r p) d -> p r d", p=P
                        ),
                    )
                    wc[name].append(w)

            dma_w("wq", wq_t, nc.sync)
            dma_w("wk", wk_t, nc.sync)
            # tiny bias loads issue after the big streams (needed ~t=30)
            nc.sync.dma_start(out=bq_sb[:], in_=bq_t[:].rearrange("(c p) -> p c", p=P))
            nc.sync.dma_start(out=bk_sb[:], in_=bk_t[:].rearrange("(c p) -> p c", p=P))
            nc.vector.tensor_scalar_mul(qbias_sb[:], bq_sb[:], 0.125)
            # mask column strips via SWDGE (gpsimd descriptor path; does not
            # contend with the shared DGE block). A dummy gpsimd read of the
            # last x chunk holds these issues back so x gets the DMA engines
            # first -- the mask is not needed until the S(0) slots.
            gate = stage.tile([1, 2], f32, name="gate", tag="gate")
            nc.gpsimd.tensor_copy(gate[:], xc[CH - 1][0:1, 0, 0:2])
            nc.gpsimd.dma_start(out=bv_bc[:], in_=bv_t[None, :].to_broadcast([P, D]))
            nc.gpsimd.dma_start(out=bo_bc[:], in_=bo_t[None, :].to_broadcast([P, D]))
            dma_w("wv", wv_t, nc.gpsimd)
            dma_w("wo", wo_t, nc.gpsimd)
            msk = []
            for kc in range(ST):
                m = stage.tile([P, ST, P], f32, name="msk", tag="msk", bufs=3)
                nc.gpsimd.dma_start(
                    out=m[:],
                    in_=mask_t[:, kc * P : (kc + 1) * P].rearrange(
                        "(i p) k -> p i k", p=P
                    ),
                )
                msk.append(m)

            # PE warm-up: real matmul work holds the HAM clock-gate open
            # while the (non-HAM-counting) transposes run
            ones512 = pp.tile([1, 512], mmdt, name="ones512", tag="ones512")
            nc.vector.memset(ones512[:], 1.0)

            def warm(n=2):
                jp = ppv.tile([P, 512], f32, name="pv", tag="pv")
                for _ in range(n):
                    nc.tensor.matmul(
                        jp[0:64, 0:512], ones_sb[:, 0:64], ones512[:],
                        start=True, stop=True,
                    )

            warm(28)

            # ---- ramp: x^T (psc ring) and wq^T/wk^T (ppv ring) ----
            xT = stage.tile([P, CH, S], mmdt, name="xT", tag="xT")
            for c in range(CH):
                ps = psc.tile([P, S], f32, name="ps", tag="ps")
                for i in range(ST):
                    nc.tensor.transpose(
                        ps[:, i * P : (i + 1) * P], xc[c][:, i, :], ident[:]
                    )
                (nc.scalar.copy if c % 2 == 0 else nc.vector.tensor_copy)(
                    xT[:, c, :], ps[:]
                )

            wT = {}

            def build_wT_chunk(name, c):
                ps = ppv.tile([P, 512], f32, name="pv", tag="pv")
                for rr in range(CH):
                    nc.tensor.transpose(
                        ps[:, rr * P : (rr + 1) * P], wc[name][c][:, rr, :], ident[:]
                    )
                (nc.scalar.copy if c % 2 == 0 else nc.vector.tensor_copy)(
                    wT[name][:, c, :], ps[:]
                )

            wT["wq"] = stage.tile([P, CH, D], mmdt, name="T", tag="Twq")
            wT["wk"] = stage.tile([P, CH, D], mmdt, name="T", tag="Twk")
            wT["wv"] = stage.tile([P, CH, D], mmdt, name="T", tag="Twv")
            wT["wo"] = pp.tile([P, CH, D], mmdt, name="T", tag="Two")
            for c in range(CH):
                build_wT_chunk("wq", c)
                build_wT_chunk("wk", c)

            # ---- projections Q^T, K^T ----
            qT = pp.tile([P, CH, S], mmdt, name="qT", tag="qT")
            kT = pp.tile([P, CH, S], mmdt, name="kT", tag="kT")

            def proj_qk_dst(c, dst, wname, bias, scale, on_act):
                ps = psc.tile([P, S], f32, name="ps", tag="ps")
                for j in range(2):
                    for rr in range(CH):
                        nc.tensor.matmul(
                            ps[:, j * 512 : (j + 1) * 512],
                            wT[wname][:, rr, c * P : (c + 1) * P],
                            xT[:, rr, j * 512 : (j + 1) * 512],
                            start=(rr == 0),
                            stop=(rr == CH - 1),
                        )
                if on_act:
                    nc.scalar.activation(
                        dst[:, c, :], ps[:], AF.Identity,
                        bias=bias[:, c : c + 1], scale=scale,
                    )
                else:
                    nc.vector.tensor_scalar(
                        dst[:, c, :], ps[:], scale, bias[:, c : c + 1],
                        op0=OP.mult, op1=OP.add,
                    )

            proj_qk_dst(0, qT, "wq", qbias_sb, 0.125, True)
            proj_qk_dst(0, kT, "wk", bk_sb, 1.0, True)

            # ---- persistent attention state ----
            omT = pp.tile([P, ST, S], mmdt, name="omT", tag="omT")
            v_sb = pp.tile([P, ST, H * 65], mmdt, name="v", tag="v")
            oT = pp.tile([P, CH, S], mmdt, name="oT", tag="oT")

            def build_om(kc):
                ps = psc.tile([P, S], f32, name="ps", tag="ps")
                for qi in range(ST):
                    nc.tensor.transpose(
                        ps[:, qi * P : (qi + 1) * P], msk[kc][:, qi, :], ident[:]
                    )
                nc.vector.tensor_scalar(
                    omT[:, kc, :], ps[:], -1.0, 1.0, op0=OP.mult, op1=OP.add
                )

            def proj_v_unit(i):
                ps = ppv.tile([P, 512], f32, name="pv", tag="pv")
                for rr in range(CH):
                    nc.tensor.matmul(
                        ps[:],
                        xT[:, rr, i * P : (i + 1) * P],
                        wT["wv"][:, rr, :],
                        start=(rr == 0),
                        stop=(rr == CH - 1),
                    )
                nc.vector.tensor_add(
                    v_sb[:, i, :].rearrange("p (h e) -> p h e", e=65)[:, :, 0:64],
                    ps[:].rearrange("p (h e) -> p h e", e=64),
                    bv_bc[:].rearrange("p (h e) -> p h e", e=64),
                )

            # partial output projection: chunks 0-2 plus bias, accumulated
            # into bf16 SBUF during S(3) so the tail needs one matmul per
            # q-tile
            f_acc = pp.tile([P, ST, 512], mmdt, name="facc", tag="facc")
            finals = []

            def partial_qt(qt):
                if qt % 2 == 0:
                    finals.append(psc.tile([P, S], f32, name="ps", tag="ps"))
                half = finals[-1][:, (qt % 2) * 512 : (qt % 2) * 512 + 512]
                for cc in range(CH - 1):
                    nc.tensor.matmul(
                        half,
                        oT[:, cc, qt * P : (qt + 1) * P],
                        wT["wo"][:, cc, :],
                        start=(cc == 0),
                        stop=(cc == CH - 2),
                    )
                nc.vector.tensor_add(f_acc[:, qt, :], half, bo_bc[:])

            # ---- pipelined attention ----
            from collections import deque

            pts = {}
            pvs = {}
            dns = {}
            pvq = {}

            def scores_unit(p, kc):
                c = p
                ptA, ptB = pts[2 * p], pts[2 * p + 1]
                kA = kT[0:64, c, kc * P : (kc + 1) * P]
                kB = kT[64:128, c, kc * P : (kc + 1) * P]
                psA = psc.tile([P, S], f32, name="ps", tag="ps")
                psB = psc.tile([P, S], f32, name="ps", tag="ps")
                for j in range(2):
                    nc.tensor.matmul(
                        psA[:, j * 512 : (j + 1) * 512],
                        kA, qT[0:64, c, j * 512 : (j + 1) * 512],
                        start=True, stop=True,
                    )
                    nc.tensor.matmul(
                        psB[:, j * 512 : (j + 1) * 512],
                        kB, qT[64:128, c, j * 512 : (j + 1) * 512],
                        start=True, stop=True,
                    )
                nc.scalar.activation(ptA[:, kc, :], psA[:], AF.Exp)
                nc.scalar.activation(ptB[:, kc, :], psB[:], AF.Exp)
                eng = nc.gpsimd if kc in GP_MUL_KC else nc.vector
                eng.tensor_mul(ptA[:, kc, :], ptA[:, kc, :], omT[:, kc, :])
                eng.tensor_mul(ptB[:, kc, :], ptB[:, kc, :], omT[:, kc, :])

            def pv_start(p, j):
                pvA = ppv.tile([P, 512], f32, name="pv", tag="pv")
                pvB = ppv.tile([P, 512], f32, name="pv", tag="pv")
                pvs[(p, j)] = (pvA, pvB)

            def pv_steps(p, j, kcs):
                hA, hB = 2 * p, 2 * p + 1
                ptA, ptB = pts[hA], pts[hB]
                vA = v_sb[:].rearrange("p i (h e) -> p i h e", e=65)[:, :, hA, :]
                vB = v_sb[:].rearrange("p i (h e) -> p i h e", e=65)[:, :, hB, :]
                jsl = slice(j * 512, (j + 1) * 512)
                pvA, pvB = pvs[(p, j)]
                for kc in kcs:
                    st = kc == 0
                    sp = kc == ST - 1
                    nc.tensor.matmul(
                        pvA[0:65, :], vA[:, kc, :], ptA[:, kc, jsl],
                        start=st, stop=sp,
                    )
                    nc.tensor.matmul(
                        pvB[0:65, :], vB[:, kc, :], ptB[:, kc, jsl],
                        start=st, stop=sp,
                    )

            def pv_drain(p, j):
                c = p
                hA, hB = 2 * p, 2 * p + 1
                jsl = slice(j * 512, (j + 1) * 512)
                pvA, pvB = pvs.pop((p, j))
                dn = dns[p]
                for idx, (h, pv) in enumerate(((hA, pvA), (hB, pvB))):
                    off = 64 * (h % 2)
                    slot = 32 * (2 * j + idx)
                    nc.vector.tensor_copy(oT[off : off + 64, c, jsl], pv[0:64, :])
                    nc.vector.tensor_copy(dn[slot : slot + 1, :], pv[64:65, :])

            def pv_norm(p, then=None):
                # one batched reciprocal for the pair's 4 denominator rows
                # (cost is free-dim-bound; non-slot lanes hold 1.0)
                c = p
                hA, hB = 2 * p, 2 * p + 1
                dn = dns.pop(p)
                rc4 = nrm.tile([P, 512], f32, name="rc4", tag="rc4")
                nc.vector.reciprocal(rc4[:], dn[:])
                rb4 = nrm.tile([P, 512], mmdt, name="rb4", tag="rb4")
                with nc.allow_low_precision(reason="bf16 recip feeds bf16 matmul"):
                    nc.vector.tensor_copy(rb4[:], rc4[:])
                for j in range(2):
                    jsl = slice(j * 512, (j + 1) * 512)
                    bp = psc.tile([P, S], f32, name="ps", tag="ps")
                    nc.tensor.matmul(
                        bp[:, 0:512], e4[:, j * P : (j + 1) * P], rb4[:],
                        start=True, stop=True,
                    )
                    for idx, h in enumerate((hA, hB)):
                        off = 64 * (h % 2)
                        osl = oT[off : off + 64, c, jsl]
                        nc.vector.tensor_mul(
                            osl, osl, bp[64 * idx : 64 * idx + 64, 0:512]
                        )
                    if then is not None:
                        then(j)

            def new_pair(p):
                pts[2 * p] = ptp.tile([P, ST, S], mmdt, name="pt", tag="pt")
                pts[2 * p + 1] = ptp.tile([P, ST, S], mmdt, name="pt", tag="pt")
                dns[p] = nrm.tile([P, 512], f32, name="dn", tag="dn")
                nc.vector.memset(dns[p][:], 1.0)
                # j-sequential: one (pair, j) accumulator group at a time
                # (the 2-deep ppv ring holds exactly one A/B group)
                pvq[p] = deque((j, kc) for j in (0, 1) for kc in range(ST))

            # ---- S(0): om + scores(0) + wv^T + V projection ----
            new_pair(0)
            nc.vector.tensor_copy(
                v_sb[:].rearrange("p i (h e) -> p i h e", e=65)[:, :, :, 64],
                ones_f32[:, 0 : ST * H].rearrange("p (i h) -> p i h", h=H),
            )
            for kc in range(ST):
                build_om(kc)
                scores_unit(0, kc)
                # chunk-1 QK projections ride the first two slots (S(1)
                # needs them; S(0) only needed chunk 0)
                if kc == 0:
                    proj_qk_dst(1, qT, "wq", qbias_sb, 0.125, True)
                if kc == 1:
                    proj_qk_dst(1, kT, "wk", bk_sb, 1.0, True)
                if kc < CH:
                    build_wT_chunk("wv", kc)
                else:
                    proj_v_unit(2 * (kc - CH))
                    proj_v_unit(2 * (kc - CH) + 1)

            # ---- S(1..3) + tail: PV-step scheduler ----
            # Each pair's PV steps (j0 then j1, kc order) are pumped into the
            # slots of the following scores phases: at slot kc of S(p), pair
            # p-1's remaining steps go first, then pair p's steps whose P^T
            # strip is already written (lag 1). ppv ring-4 holds at most two
            # (pair, j) accumulator groups at a time.
            started = set()
            jdone = {}

            def pump(order, avail, budget):
                for p_, limit in order:
                    q = pvq.get(p_)
                    if p_ > 0 and (p_ - 1) in pvq:
                        continue  # previous pair still owns the ppv ring
                    while q and budget > 0 and limit > 0:
                        j, kc = q[0]
                        if kc > avail.get(p_, ST):
                            break
                        q.popleft()
                        if (p_, j) not in started:
                            started.add((p_, j))
                            pv_start(p_, j)
                        pv_steps(p_, j, [kc])
                        jdone[(p_, j)] = jdone.get((p_, j), 0) + 1
                        if jdone[(p_, j)] == ST:
                            pv_drain(p_, j)
                            if jdone.get((p_, 1)) == ST:
                                del pvq[p_]
                                # pair 3's norm is interleaved with the
                                # output projection in the tail
                                if p_ != H // 2 - 1:
                                    pv_norm(p_)
                                break
                        budget -= 1
                        limit -= 1

            bg = {
                (1, 3): lambda: proj_qk_dst(2, qT, "wq", qbias_sb, 0.125, False),
                (1, 6): lambda: proj_qk_dst(2, kT, "wk", bk_sb, 1.0, False),
                (2, 0): lambda: build_wT_chunk("wo", 0),
                (2, 2): lambda: proj_qk_dst(3, qT, "wq", qbias_sb, 0.125, False),
                (2, 3): lambda: build_wT_chunk("wo", 1),
                (2, 5): lambda: proj_qk_dst(3, kT, "wk", bk_sb, 1.0, False),
                (2, 6): lambda: build_wT_chunk("wo", 2),
                (3, 0): lambda: build_wT_chunk("wo", 3),
            }

            for p in (1, 2, 3):
                new_pair(p)
                for kc in range(ST):
                    scores_unit(p, kc)
                    if (p, kc) in bg:
                        bg[(p, kc)]()
                    # front-load: finish pair p-1's PV a slot early so its
                    # last step doesn't collide with the pt-ring recycle at
                    # the pair boundary
                    n = 3 if kc < 2 else 2
                    pump([(p - 1, n)], {p - 1: ST}, n)

            # tail: pair-3 PV pumped in chunks, interleaved with the
            # partial output projections (chunks 0-2 are normalized by now)
            for qtp in range(4):
                pump([(3, 4)], {3: ST}, 4)
                partial_qt(2 * qtp)
                partial_qt(2 * qtp + 1)
            while pvq:
                pump([(0, 8), (1, 8), (2, 8), (3, 8)], {0: ST, 1: ST, 2: ST, 3: ST}, 16)

            warm(8)

            def tail_finals(j):
                # oT chunk 3 cols j*512.. cover q-tiles 4j..4j+3: one matmul
                # each on top of the precomputed partial
                for qt in range(4 * j, 4 * j + 4):
                    if qt % 2 == 0:
                        finals.append(psc.tile([P, S], f32, name="ps", tag="ps"))
                    half = finals[-1][:, (qt % 2) * 512 : (qt % 2) * 512 + 512]
                    nc.tensor.matmul(
                        half,
                        oT[:, 3, qt * P : (qt + 1) * P],
                        wT["wo"][:, 3, :],
                        start=True, stop=True,
                    )
                    ft = fpool.tile([P, 512], f32, name="fin", tag="fin")
                    nc.vector.tensor_add(ft[:], half, f_acc[:, qt, :])
                    # alternate output DMAs across the SP and (idle) ACT
                    # queues to halve tail issue serialization
                    eng = nc.sync if qt % 2 == 0 else nc.scalar
                    eng.dma_start(out=out_t[qt * P : (qt + 1) * P, :], in_=ft[:])

            pv_norm(3, then=tail_finals)

    nc.compile()
    return nc


def _get_nc():
    if "nc" not in _CACHE:
        _CACHE["nc"] = _build()
    return _CACHE["nc"]


def run(inputs, trace=False, **kw):
    from concourse.bass_utils import run_bass_kernel_spmd

    nc = _get_nc()
    f = np.float32
    in_maps = [
        {
            "x": np.ascontiguousarray(inputs["inputs"][i], dtype=f),
            "mask": np.ascontiguousarray(inputs["mask"][i], dtype=f),
            "wq": np.ascontiguousarray(inputs["wq"], dtype=f),
            "wk": np.ascontiguousarray(inputs["wk"], dtype=f),
            "wv": np.ascontiguousarray(inputs["wv"], dtype=f),
            "wo": np.ascontiguousarray(inputs["wo"], dtype=f),
            "bq": np.ascontiguousarray(inputs["bq"], dtype=f),
            "bk": np.ascontiguousarray(inputs["bk"], dtype=f),
            "bv": np.ascontiguousarray(inputs["bv"], dtype=f),
            "bo": np.ascontiguousarray(inputs["bo"], dtype=f),
        }
        for i in range(NCORES)
    ]
    res = run_bass_kernel_spmd(nc, in_maps, list(range(NCORES)), trace=trace, **kw)
    out = np.stack(
        [np.asarray(res.results[i]["out"], dtype=np.float32) for i in range(NCORES)],
        axis=0,
    )
    return out, res


def kernel(**inputs):
    out, _ = run(inputs)
    return out
